# revision 1
# baseline (speedup 1.0000x reference)
"""AttentionDCA energy kernel for 8 Trainium2 NeuronCores (fp8 DoubleRow).

Math: with one-hot E_b in {0,1}^{L x 21} for sequence x[b],
    energy[b] = -sum_h <E_b^T A_h E_b, V_h>_F
where A_h = softmax(Q_h K_h^T / d_k).  Everything becomes PE matmuls;
phase 2 runs fp8e4m3 DoubleRow (halved instruction count), the softmax
numerators and one-hots travel as fp8, the normalized row-blocks as bf16
(fp8 R costs ~2.7e-2 relative error; bf16 keeps it at ~3.5e-3):

  per core (4 heads, H sharded over 8 cores):
    S_T[j,i]   = K_h Q_h^T                  (fp8 scores, transposed layout)
    ex         = exp(S_T / d_k)             (ScalarE, PSUM->SBUF fp8)
    R'[i,col]  = sum_j ex[j,i] * EA[j,col]  (PE DoubleRow; EA = one-hot for all
                                             64 b + a 1/16-column -> r/16)
    r_hat      = R' * (16/r[i])             (per-chunk evictions split across
                                             ScalarE/VectorE into bf16; the
                                             1/16 ones-column makes the DVE
                                             reciprocal directly yield 16/r)
    C          = E_g^T r_hat_g              (PE, 6 b's block-diagonal batched
                                             over 11 groups, N=504)
    S[p,g]     = sum((C * -1/16) * V)       (one fused DVE scalar_tensor_tensor
                                             with free-axis accumulate)
    energy     = P6^T S                     (PE selector matmul, partition sum)

Host only shards/stages inputs (fp8/bf16) and sums the 8 partial [6,11]
outputs.  PE warm-up matmuls on zeroed tiles bridge the input-DMA window
so the HAM clock gate releases (2.4 GHz) before real matmuls start.
Each 512-col score chunk accumulates in its own PSUM bank with its own
eviction op, so pr slots free chunk-by-chunk and the eviction chain
(reciprocal -> evict) hides under the remaining DoubleRow matmuls.
"""

import numpy as np
import ml_dtypes

# Problem constants (hardcoded per contract)
B, L, H, D, NAA = 64, 512, 32, 128, 21
NCORES = 8
HPC = H // NCORES            # heads per core = 4
JB = L // 128                # 4 position blocks
BG = 6                       # b's per group
NG = (B + BG - 1) // BG      # 11 groups
BPAD = BG * NG               # 66 padded batch
NDATA = BPAD * NAA           # 1386 one-hot columns (64 real b's + 2 zero pad)
ONES_COL = NDATA             # 1386: 1/16-column -> r/16 row sums
NEA = 1392                   # eb width: 1386 data + ones col + 5 zero pad
MP = BG * NAA                # 126 used output partitions of mat2
GW = 128                     # group stride in the mat2 E copy
NE2 = NG * GW                # 1408 columns of the mat2 E copy
NQ = HPC * MP                # 504 output free size of mat2: also the phase-2
CW = NQ                      # chunk width (504, 504, 378+ones over 3 banks),
CW2 = NDATA - 2 * CW         # 378: so chunks align with 126-col groups
RROW = NG * NQ               # 5544 used + pad to 5568 (16-aligned DR stride)
RPAD = 5568
NWARM = 4                    # PE warm-up matmuls during the input DMA window:
                             # qk0 lands ~2.2us after the first warm-up MM
                             # (traced), so 4 cold ~550ns matmuls end right
                             # as the first scores' data arrives; 7 queued
                             # the ready scores ~2us behind surplus warm-ups

_NC = None


def _build_nc():
    import concourse.bacc as bacc
    import concourse.tile as tile
    from concourse import mybir

    f32 = mybir.dt.float32
    bf16 = mybir.dt.bfloat16
    fp8 = mybir.dt.float8e4
    AF = mybir.ActivationFunctionType
    DR = mybir.MatmulPerfMode.DoubleRow
    mult = mybir.AluOpType.mult

    nc = bacc.Bacc("TRN2", target_bir_lowering=False, debug=False)

    qk0_d = nc.dram_tensor("qk0", [128, 2, 2, L], fp8, kind="ExternalInput")
    qk1_d = nc.dram_tensor("qk1", [128, 2, 2, L], fp8, kind="ExternalInput")
    eb_d = nc.dram_tensor("eb", [128, JB, NEA], fp8, kind="ExternalInput")
    e2_d = nc.dram_tensor("e2", [128, JB, NE2], fp8, kind="ExternalInput")
    vv_d = nc.dram_tensor("vv", [128, NQ], bf16, kind="ExternalInput")
    p6_d = nc.dram_tensor("p6", [128, BG], f32, kind="ExternalInput")
    out_d = nc.dram_tensor("energy", [BG, NG], f32, kind="ExternalOutput")

    with tile.TileContext(nc) as tc:
        with (
            tc.tile_pool(name="const", bufs=1) as cpool,
            tc.tile_pool(name="exps", bufs=HPC) as xpool,
            tc.tile_pool(name="rall", bufs=1) as rpool,
            tc.tile_pool(name="small", bufs=8) as spool,
            tc.tile_pool(name="psum", bufs=2, space="PSUM") as pp,
        ):
            qk0_sb = cpool.tile([128, 2, 2, L], fp8, tag="qk0")
            qk1_sb = cpool.tile([128, 2, 2, L], fp8, tag="qk1")
            eb_sb = cpool.tile([128, JB, NEA], fp8, tag="eb")
            e2_sb = cpool.tile([128, JB, NE2], fp8, tag="e2")
            vv_sb = cpool.tile([128, NQ], bf16, tag="vv")
            p6_sb = cpool.tile([128, BG], f32, tag="p6")
            s_sb = cpool.tile([128, NG], f32, tag="ssb")
            zero_sb = cpool.tile([128, 1], f32, tag="zero")
            wz_l = cpool.tile([128, 128], fp8, tag="wzl")
            wz_r = cpool.tile([128, L], fp8, tag="wzr")

            # qk0 gates the first scores; eb (needed by mat1 ~2us later)
            # goes second; e2/vv/p6 are phase-3 inputs and can trail
            nc.sync.dma_start(out=qk0_sb[:], in_=qk0_d[:])
            nc.sync.dma_start(out=eb_sb[:], in_=eb_d[:])
            nc.sync.dma_start(out=qk1_sb[:], in_=qk1_d[:])
            nc.sync.dma_start(out=vv_sb[:], in_=vv_d[:])
            nc.sync.dma_start(out=p6_sb[:], in_=p6_d[:])
            nc.sync.dma_start(out=e2_sb[:], in_=e2_d[:])
            nc.vector.memset(zero_sb[:], 0.0)
            nc.vector.memset(wz_l[:], 0.0)
            nc.vector.memset(wz_r[:], 0.0)

            # PE warm-up: matmuls on zeroed tiles with no DMA dependency keep
            # the PE busy through the input-DMA window so HAM un-throttles
            # before real work, and the first scores aren't run at 1.2 GHz
            for _ in range(NWARM):
                pw = pp.tile([128, L], f32, tag="small")
                nc.tensor.matmul(pw[:], wz_l[:], wz_r[:], start=True, stop=True)

            # Phase 1+2 interleaved: scores+exp for head h are emitted two
            # heads ahead of mat1(h), so PE streams score matmuls for h+2
            # while ScalarE runs exp(h+1) and PE's mat1(h) chews on exp(h).
            exps = []

            def scores(h):
                ex = xpool.tile([128, JB, L], fp8, tag="ex")
                exps.append(ex)
                for jb in range(JB):
                    # borrow the (still idle) pr psum slots for half the
                    # first head's score tiles so the 4-matmul chain isn't
                    # gated on exp; later heads run beside mat1, whose pr
                    # tiles need those slots
                    ps = pp.tile([128, L], f32,
                                 tag="small" if (jb < 2 or h >= 1) else "big")
                    qq = qk0_sb if h < 2 else qk1_sb
                    nc.tensor.matmul(
                        ps[:],
                        qq[:, 1, h % 2, jb * 128:(jb + 1) * 128],
                        qq[:, 0, h % 2, :],
                        start=True,
                        stop=True,
                    )
                    nc.scalar.activation(
                        ex[:, jb, :], ps[:], AF.Exp,
                        bias=zero_sb[:], scale=1.0 / D,
                    )

            scores(0)
            scores(1)
            # r_hat layout: [p, ib, (g, h, a)] so each group's 504 phase-3
            # columns are contiguous
            r_sb = rpool.tile([128, JB, RPAD], bf16, tag="r")

            for h in range(HPC):
                ex = exps[h]
                for ib in range(JB):
                    pr = pp.tile([128, 3, 512], f32, tag="big")
                    lhs0 = ex[:, 0:2, ib * 128:(ib + 1) * 128]
                    lhs1 = ex[:, 2:4, ib * 128:(ib + 1) * 128]
                    # DoubleRow: contract j in two 256-row steps; at the last
                    # step finish chunk 2 first: the 1/16-ones column lives
                    # there, so the reciprocal overlaps the remaining matmuls
                    steps = ((0, lhs0, (0, 1, 2)), (1, lhs1, (2, 0, 1)))
                    for s, lhs, cks in steps:
                        for ck in cks:
                            w = CW2 + 1 if ck == 2 else CW
                            nc.tensor.matmul(
                                pr[:, ck, :w],
                                lhs,
                                eb_sb[:, 2 * s:2 * s + 2,
                                      ck * CW:ck * CW + w],
                                start=(s == 0),
                                stop=(s == 1),
                                perf_mode=DR,
                            )
                    rcp16 = spool.tile([128, 1], f32, tag="rcp")
                    # ones column holds 1/16 -> accumulated r/16 -> 16/r here
                    nc.vector.reciprocal(rcp16[:], pr[:, 2, CW2:CW2 + 1])
                    # evict (PSUM f32 -> SBUF bf16, scale 16/r) into the
                    # (g, h, a) layout, split: VectorE banks 0-1, ScalarE
                    # bank 2 -> r_hat = 16 * softmax row block (the swapped
                    # assignment measured 1.5us slower: ScalarE's exp chain
                    # can't absorb the long banks-0-1 op)
                    rg = r_sb[:, ib, :RROW].rearrange(
                        "p (g h w) -> p g h w", g=NG, h=HPC)
                    out01 = rg[:, 0:8, h, :].rearrange(
                        "p (c g) w -> p c g w", c=2)
                    in01 = pr[:, 0:2, 0:CW].rearrange(
                        "p c (g w) -> p c g w", g=4)
                    out2 = rg[:, 8:11, h, :]
                    in2 = pr[:, 2, 0:CW2].rearrange("p (g w) -> p g w", g=3)
                    nc.vector.tensor_scalar_mul(out01, in01, rcp16[:])
                    nc.scalar.mul(out2, in2, rcp16[:])
                if h + 2 < HPC:
                    scores(h + 2)

            # Phase 3: C = E^T r_hat (block-diagonal over 6 b's, bf16 moving,
            # fp8 stationary), then one fused DVE multiply-by-V with free-axis
            # accumulate per group
            for g in range(NG):
                # alternate psum tags so 4 groups are in flight and the PE
                # never waits on the trailing DVE reduce of group g-2
                if g % 2 == 0:
                    pct = pp.tile([128, NQ], f32, tag="small")
                    pc = pct[:]
                else:
                    pct = pp.tile([128, 3, 512], f32, tag="big")
                    pc = pct[:, 0, 0:NQ]
                for ib in range(JB):
                    nc.tensor.matmul(
                        pc,
                        e2_sb[:, ib, g * GW:(g + 1) * GW],
                        r_sb[:, ib, g * NQ:(g + 1) * NQ],
                        start=(ib == 0),
                        stop=(ib == JB - 1),
                    )
                scr = spool.tile([128, NQ], bf16, tag="scr")
                nc.vector.scalar_tensor_tensor(
                    out=scr[:],
                    in0=pc,
                    scalar=-1.0 / 16.0,
                    in1=vv_sb[:],
                    op0=mult,
                    op1=mult,
                    accum_out=s_sb[:, g:g + 1],
                )

            # Phase 4: cross-partition sum via selector matmul
            pe = pp.tile([BG, NG], f32, tag="small")
            nc.tensor.matmul(pe[:], p6_sb[:], s_sb[:], start=True, stop=True)
            eout = spool.tile([BG, NG], f32, tag="eout")
            nc.scalar.copy(eout[:], pe[:])
            nc.sync.dma_start(out=out_d[:], in_=eout[:])

    nc.compile()
    return nc


def _get_nc():
    global _NC
    if _NC is None:
        _NC = _build_nc()
    return _NC


def _stage_inputs(x, Q, K, V):
    """Host-side sharding/staging. Returns in_maps for the 8 cores."""
    fp8 = ml_dtypes.float8_e4m3
    bf16 = ml_dtypes.bfloat16
    x = np.asarray(x)
    Q = np.asarray(Q, dtype=np.float32)
    K = np.asarray(K, dtype=np.float32)
    V = np.asarray(V, dtype=np.float32)

    # One-hot EA [L, NEA] (+ 1/16 column at 1386), replicated to all cores
    onehot = (x[:, :, None] == np.arange(NAA, dtype=x.dtype)[None, None, :])
    ea = np.zeros((L, NEA), dtype=np.float32)
    ea[:, : B * NAA] = onehot.transpose(1, 0, 2).reshape(L, B * NAA)
    ea[:, ONES_COL] = 1.0 / 16.0
    eb_host = np.ascontiguousarray(
        ea.reshape(JB, 128, NEA).transpose(1, 0, 2)
    ).astype(fp8)

    ea2 = np.zeros((L, NE2), dtype=np.float32)
    for g in range(NG):
        nb = min(BG, B - g * BG)
        blk = onehot[g * BG: g * BG + nb].transpose(1, 0, 2).reshape(L, nb * NAA)
        ea2[:, g * GW: g * GW + nb * NAA] = blk
    e2_host = np.ascontiguousarray(
        ea2.reshape(JB, 128, NE2).transpose(1, 0, 2)
    ).astype(fp8)

    p6 = np.zeros((128, BG), dtype=np.float32)
    for bl in range(BG):
        p6[bl * NAA:(bl + 1) * NAA, bl] = 1.0

    in_maps = []
    for c in range(NCORES):
        hs = slice(c * HPC, (c + 1) * HPC)
        qt = Q[hs].transpose(2, 0, 1)
        kt = K[hs].transpose(2, 0, 1)
        qk0 = np.ascontiguousarray(
            np.stack([qt[:, 0:2], kt[:, 0:2]], axis=1)).astype(fp8)
        qk1 = np.ascontiguousarray(
            np.stack([qt[:, 2:4], kt[:, 2:4]], axis=1)).astype(fp8)
        vv = np.zeros((128, NQ), dtype=np.float32)
        vc = V[hs]
        for h in range(HPC):
            for bl in range(BG):
                vv[bl * NAA:(bl + 1) * NAA,
                   h * MP + bl * NAA: h * MP + (bl + 1) * NAA] = vc[h]
        in_maps.append({"qk0": qk0, "qk1": qk1, "eb": eb_host, "e2": e2_host,
                        "vv": vv.astype(bf16), "p6": p6})
    return in_maps


def _run(x, Q, K, V, trace=False):
    from concourse.bass_utils import run_bass_kernel_spmd

    nc = _get_nc()
    in_maps = _stage_inputs(x, Q, K, V)
    res = run_bass_kernel_spmd(nc, in_maps, list(range(NCORES)), trace=trace)

    total = np.zeros((BG, NG), dtype=np.float64)
    for r in res.results:
        total += r["energy"].astype(np.float64)
    bidx = np.arange(B)
    energy = total[bidx % BG, bidx // BG].astype(np.float32)
    return energy, res


def kernel(x, Q, K, V):
    return _run(x, Q, K, V)[0]



# revision 2
# speedup vs baseline: 1.0622x; 1.0622x over previous
"""AttentionDCA energy kernel for 8 Trainium2 NeuronCores (fp8 DoubleRow).

Math: with one-hot E_b in {0,1}^{L x 21} for sequence x[b],
    energy[b] = -sum_h <E_b^T A_h E_b, V_h>_F
where A_h = softmax(Q_h K_h^T / d_k).  Everything becomes PE matmuls;
phase 2 runs fp8e4m3 DoubleRow (halved instruction count), the softmax
numerators and one-hots travel as fp8, the normalized row-blocks as bf16.

  per core (4 heads, H sharded over 8 cores):
    S_T[j,i]   = K_h Q_h^T                  (fp8 scores, transposed layout)
    ex         = exp(S_T / d_k)             (ScalarE, PSUM->SBUF fp8)
    R'[i,col]  = sum_j ex[j,i] * EA[j,col]  (PE DoubleRow; EA = one-hot for all
                                             64 b + a 1/16-column -> r/16)
    r_hat      = R' * (16/r[i])             (ScalarE evicts the ones-chunk,
                                             VectorE the two 504-col chunks,
                                             both into a contiguous h-major
                                             bf16 row block)
    C          = E_g^T r_hat_g              (PE; stationary = a 128-col window
                                             of EA itself -> FWL, no separate
                                             E2 copy)
    S[p,g]     = sum((C * -1/16) * V)       (one fused DVE scalar_tensor_tensor
                                             with free-axis accumulate)
  host: 21-row segment sums of the [128, 11] per-core S, summed over cores.

v2 notes vs v1 (traced):
  - input DMAs are chained (qk0 -> {eb, qk1} -> vv) so qk0 gets full HBM
    bandwidth and the first score matmul starts ~1us earlier.
  - phase-2 ones-chunk (379 cols incl the 1/16 column) accumulates FIRST in
    its own 1-bank psum tile, so reciprocal + ScalarE eviction overlap the
    remaining 2-bank DoubleRow matmuls instead of gating the next iteration.
  - r_hat rows are h-major and each eviction is contiguous (no 4-D rearrange
    APs on the DVE).
  - phase 3 loads its stationary from eb directly (group g = cols
    126g..126g+128; the 2 overhang cols only feed vv rows 126/127 which are
    zero) and streams r_hat via a strided [4h x 126] AP.
  - phase 4 (selector matmul + copy) is gone: the [128, 11] accumulator DMAs
    out and the 21-row sums happen on host with the cross-core reduce.
"""

import numpy as np
import ml_dtypes

# Problem constants (hardcoded per contract)
B, L, H, D, NAA = 64, 512, 32, 128, 21
NCORES = 8
HPC = H // NCORES            # heads per core = 4
JB = L // 128                # 4 position blocks
BG = 6                       # b's per group
NG = (B + BG - 1) // BG      # 11 groups
BPAD = BG * NG               # 66 padded batch
NDATA = BPAD * NAA           # 1386 one-hot columns (64 real b's + 2 zero pad)
ONES_COL = NDATA             # 1386: 1/16-column -> r/16 row sums
NEA = 1392                   # eb width: 1386 data + ones col + 5 zero pad
MP = BG * NAA                # 126 output partitions used per group
GW = MP                      # 126: group stride in eb's column space
NQ = HPC * MP                # 504: phase-3 free size (h-major)
CW = 504                     # phase-2 chunk width for the two full banks
CW2 = NDATA - 2 * CW         # 378 data cols in the ones chunk
RW = 1392                    # per-(ib,h) r_hat row width (1386 + pad)
NWARM = 4                    # PE warm-up matmuls briding the qk0 DMA window

_NC = None


def _build_nc():
    import concourse.bacc as bacc
    import concourse.tile as tile
    from concourse import mybir
    from concourse.tile_rust import add_dep_helper

    f32 = mybir.dt.float32
    bf16 = mybir.dt.bfloat16
    fp8 = mybir.dt.float8e4
    AF = mybir.ActivationFunctionType
    DR = mybir.MatmulPerfMode.DoubleRow
    mult = mybir.AluOpType.mult

    nc = bacc.Bacc("TRN2", target_bir_lowering=False, debug=False)

    qk0_d = nc.dram_tensor("qk0", [128, 2, 2, L], fp8, kind="ExternalInput")
    qk1_d = nc.dram_tensor("qk1", [128, 2, 2, L], fp8, kind="ExternalInput")
    eb_d = nc.dram_tensor("eb", [128, JB, NEA], fp8, kind="ExternalInput")
    vv_d = nc.dram_tensor("vv", [128, NQ], bf16, kind="ExternalInput")
    out_d = nc.dram_tensor("energy", [128, NG], f32, kind="ExternalOutput")

    with tile.TileContext(nc) as tc:
        with (
            tc.tile_pool(name="const", bufs=1) as cpool,
            tc.tile_pool(name="exps", bufs=HPC) as xpool,
            tc.tile_pool(name="rall", bufs=1) as rpool,
            tc.tile_pool(name="small", bufs=8) as spool,
            tc.tile_pool(name="ps", bufs=2, space="PSUM") as ps_pool,
            tc.tile_pool(name="pr1", bufs=2, space="PSUM") as pr1_pool,
            tc.tile_pool(name="pr2", bufs=2, space="PSUM") as pr2_pool,
        ):
            qk0_sb = cpool.tile([128, 2, 2, L], fp8, tag="qk0")
            qk1_sb = cpool.tile([128, 2, 2, L], fp8, tag="qk1")
            eb_sb = cpool.tile([128, JB, NEA], fp8, tag="eb")
            vv_sb = cpool.tile([128, NQ], bf16, tag="vv")
            s_sb = cpool.tile([128, NG], f32, tag="ssb")
            wz_l = cpool.tile([128, 128], fp8, tag="wzl")
            wz_r = cpool.tile([128, L], fp8, tag="wzr")

            # qk0 gates the first scores: give it the full DMA bandwidth,
            # then eb (mat1) + qk1 (scores 2/3), then vv (phase 3 only)
            d0 = nc.sync.dma_start(out=qk0_sb[:], in_=qk0_d[:])
            d1 = nc.sync.dma_start(out=eb_sb[:], in_=eb_d[:])
            d2 = nc.sync.dma_start(out=qk1_sb[:], in_=qk1_d[:])
            d3 = nc.sync.dma_start(out=vv_sb[:], in_=vv_d[:])
            add_dep_helper(d1.ins, d0.ins, sync=True, reason="qk0 DMA first")
            add_dep_helper(d2.ins, d0.ins, sync=True, reason="qk0 DMA first")
            add_dep_helper(d3.ins, d2.ins, sync=True, reason="vv DMA last")
            nc.vector.memset(wz_l[:], 0.0)
            nc.vector.memset(wz_r[:], 0.0)

            # PE warm-up: matmuls on zeroed tiles with no DMA dependency keep
            # the PE busy through the qk0-DMA window so HAM un-throttles
            # before real work
            for _ in range(NWARM):
                pw = ps_pool.tile([128, 512], f32, tag="ps")
                nc.tensor.matmul(pw[:], wz_l[:], wz_r[:], start=True, stop=True)

            # Phase 1+2 interleaved: scores+exp for head h are emitted two
            # heads ahead of mat1(h), so PE streams score matmuls for h+2
            # while ScalarE runs exp(h+1) and PE's mat1(h) chews on exp(h).
            exps = []

            def scores(h):
                ex = xpool.tile([128, JB, L], fp8, tag="ex")
                exps.append(ex)
                qq = qk0_sb if h < 2 else qk1_sb
                for jb in range(JB):
                    # borrow the (still idle) pr2 psum banks for half the
                    # first head's score tiles so the 4-matmul chain isn't
                    # gated on exp recycling the 2 ps bufs
                    if h == 0 and jb >= 2:
                        pst = pr2_pool.tile([128, 2, 512], f32, tag="pr2")
                        psc = pst[:, jb - 2, :]
                    else:
                        pst = ps_pool.tile([128, 512], f32, tag="ps")
                        psc = pst[:]
                    nc.tensor.matmul(
                        psc,
                        qq[:, 1, h % 2, jb * 128:(jb + 1) * 128],
                        qq[:, 0, h % 2, :],
                        start=True,
                        stop=True,
                    )
                    nc.scalar.activation(
                        ex[:, jb, :], psc, AF.Exp, scale=1.0 / D,
                    )

            scores(0)
            scores(1)
            # r_hat layout: [p, ib, h, col] with each head's 1386 columns
            # contiguous, so every eviction is a contiguous store
            r_sb = rpool.tile([128, JB, HPC, RW], bf16, tag="r")

            for h in range(HPC):
                ex = exps[h]
                for ib in range(JB):
                    lhs0 = ex[:, 0:2, ib * 128:(ib + 1) * 128]
                    lhs1 = ex[:, 2:4, ib * 128:(ib + 1) * 128]
                    steps = ((0, lhs0), (1, lhs1))
                    # ones-chunk first, in its own 1-bank tile: reciprocal +
                    # ScalarE eviction run under the remaining 4 matmuls
                    pa = pr1_pool.tile([128, 512], f32, tag="pr1")
                    for s, lhs in steps:
                        nc.tensor.matmul(
                            pa[:, 0:CW2 + 1],
                            lhs,
                            eb_sb[:, 2 * s:2 * s + 2, 2 * CW:2 * CW + CW2 + 1],
                            start=(s == 0),
                            stop=(s == 1),
                            perf_mode=DR,
                        )
                    rcp16 = spool.tile([128, 1], f32, tag="rcp")
                    # ones column holds 1/16 -> accumulated r/16 -> 16/r here
                    nc.vector.reciprocal(rcp16[:], pa[:, CW2:CW2 + 1])
                    pb = pr2_pool.tile([128, 2, 512], f32, tag="pr2")
                    for ck in (0, 1):
                        for s, lhs in steps:
                            nc.tensor.matmul(
                                pb[:, ck, 0:CW],
                                lhs,
                                eb_sb[:, 2 * s:2 * s + 2, ck * CW:(ck + 1) * CW],
                                start=(s == 0),
                                stop=(s == 1),
                                perf_mode=DR,
                            )
                    # evictions (PSUM f32 -> SBUF bf16, scale 16/r):
                    # ScalarE takes the ones chunk, VectorE the two full banks
                    nc.scalar.mul(
                        r_sb[:, ib, h, 2 * CW:2 * CW + CW2],
                        pa[:, 0:CW2],
                        rcp16[:],
                    )
                    nc.vector.tensor_scalar_mul(
                        r_sb[:, ib, h, 0:2 * CW].rearrange(
                            "p (c w) -> p c w", c=2),
                        pb[:, :, 0:CW],
                        rcp16[:],
                    )
                if h + 2 < HPC:
                    scores(h + 2)

            # Phase 3: C = E_g^T r_hat (bf16 moving via a [4h x 126] strided
            # AP, stationary = 128-col window of eb), then one fused DVE
            # multiply-by-V with free-axis accumulate per group
            pool_cycle = (ps_pool, pr1_pool, pr2_pool)
            for g in range(NG):
                pool = pool_cycle[g % 3]
                if pool is pr2_pool:
                    pct = pool.tile([128, 2, 512], f32, tag="pr2")
                    pc = pct[:, 0, 0:NQ]
                elif pool is pr1_pool:
                    pct = pool.tile([128, 512], f32, tag="pr1")
                    pc = pct[:, 0:NQ]
                else:
                    pct = pool.tile([128, 512], f32, tag="ps")
                    pc = pct[:, 0:NQ]
                for ib in range(JB):
                    nc.tensor.matmul(
                        pc,
                        eb_sb[:, ib, GW * g:GW * g + 128],
                        r_sb[:, ib, :, GW * g:GW * g + GW],
                        start=(ib == 0),
                        stop=(ib == JB - 1),
                    )
                scr = spool.tile([128, NQ], bf16, tag="scr")
                nc.vector.scalar_tensor_tensor(
                    out=scr[:],
                    in0=pc,
                    scalar=-1.0 / 16.0,
                    in1=vv_sb[:],
                    op0=mult,
                    op1=mult,
                    accum_out=s_sb[:, g:g + 1],
                )

            nc.sync.dma_start(out=out_d[:], in_=s_sb[:])

    nc.compile()
    return nc


def _get_nc():
    global _NC
    if _NC is None:
        _NC = _build_nc()
    return _NC


def _stage_inputs(x, Q, K, V):
    """Host-side sharding/staging. Returns in_maps for the 8 cores."""
    fp8 = ml_dtypes.float8_e4m3
    bf16 = ml_dtypes.bfloat16
    x = np.asarray(x)
    Q = np.asarray(Q, dtype=np.float32)
    K = np.asarray(K, dtype=np.float32)
    V = np.asarray(V, dtype=np.float32)

    # One-hot EA [L, NEA] (+ 1/16 column at 1386), replicated to all cores
    onehot = (x[:, :, None] == np.arange(NAA, dtype=x.dtype)[None, None, :])
    ea = np.zeros((L, NEA), dtype=np.float32)
    ea[:, : B * NAA] = onehot.transpose(1, 0, 2).reshape(L, B * NAA)
    ea[:, ONES_COL] = 1.0 / 16.0
    eb_host = np.ascontiguousarray(
        ea.reshape(JB, 128, NEA).transpose(1, 0, 2)
    ).astype(fp8)

    in_maps = []
    for c in range(NCORES):
        hs = slice(c * HPC, (c + 1) * HPC)
        qt = Q[hs].transpose(2, 0, 1)
        kt = K[hs].transpose(2, 0, 1)
        qk0 = np.ascontiguousarray(
            np.stack([qt[:, 0:2], kt[:, 0:2]], axis=1)).astype(fp8)
        qk1 = np.ascontiguousarray(
            np.stack([qt[:, 2:4], kt[:, 2:4]], axis=1)).astype(fp8)
        vv = np.zeros((128, NQ), dtype=np.float32)
        vc = V[hs]
        for h in range(HPC):
            for bl in range(BG):
                vv[bl * NAA:(bl + 1) * NAA,
                   h * MP + bl * NAA: h * MP + (bl + 1) * NAA] = vc[h]
        in_maps.append({"qk0": qk0, "qk1": qk1, "eb": eb_host,
                        "vv": vv.astype(bf16)})
    return in_maps


def _reduce_energy(arr):
    """[128, NG] per-core accumulator -> [BG, NG] via 21-row segment sums."""
    return arr[:MP].reshape(BG, NAA, NG).sum(axis=1)


def _run(x, Q, K, V, trace=False):
    from concourse.bass_utils import run_bass_kernel_spmd

    nc = _get_nc()
    in_maps = _stage_inputs(x, Q, K, V)
    res = run_bass_kernel_spmd(nc, in_maps, list(range(NCORES)), trace=trace)

    total = np.zeros((BG, NG), dtype=np.float64)
    for r in res.results:
        total += _reduce_energy(r["energy"].astype(np.float64))
    bidx = np.arange(B)
    energy = total[bidx % BG, bidx // BG].astype(np.float32)
    return energy, res


def kernel(x, Q, K, V):
    return _run(x, Q, K, V)[0]
